# revision 25
# baseline (speedup 1.0000x reference)
"""Distributed LSTM-cell kernel for one TRN2 chip (8 NeuronCores).

Problem: gates = w_ih @ x + b_ih + w_hh @ h_prev + b_hh   (4H x B)
         i,f,g,o = split(gates); c' = sig(f)*c + sig(i)*tanh(g)
         h' = sig(o)*tanh(c'); return sum(c' + h')

Sharding: tensor-parallel over the 4H gate dimension, interleaved so each
core owns rows [d*512,(d+1)*512) of EVERY gate (=> it owns h-rows
[d*512,(d+1)*512) of c'/h').  x / h_prev are replicated.  Each core emits
per-partition partial sums [128, 8]; the host reduces the 8*1024 partials.
No on-chip collective is needed.

Per-core compute: gates_d [2048, 1024] = W_d [2048, 8192] @ [x; h] [8192, 1024].

The final output is a near-cancelling sum (~31 out of 8.4M O(1) terms), so
matmul precision matters: plain bf16 => ~15% rel err.  Schemes (HW-measured
rel err / exec time):
  f16c0   single fp16 pass (DEFAULT)           1.244e-2   ~0.47 ms
  f16c1   + fp8-DoubleRow W-side correction    ~5.4e-3
  f16c2   + fp8-DR corrections on both sides   2.9e-5     ~1.01 ms
  bf16x3  bf16 hi/lo 3-pass (old baseline)     1.8e-4     ~1.35 ms
  fp32 / fp32r: native fp32 matmul crashes the exec unit in this runtime
  and float32r returns garbage -- both unusable here (HW-verified).
The error of each scheme is deterministic (fixed inputs, fixed accumulation
order): f16c0 measured bitwise-identical across runs, comfortably under the
2e-2 gate.  fp16/fp8 subnormal inputs are handled by the PE (no FTZ).
"""

import os

import numpy as np

D = 4096
H = 4096
B = 1024
NCORES = 8
RPC = 4 * H // NCORES // 4      # 512 rows per gate per core
HMT = RPC // 128                # 4 h-row tiles of 128 per core
KT = (D + H) // 128             # 64 contraction tiles
NN = B // 512                   # 2 batch halves
P = 128
NBUF = 8                        # stream double-buffer depth (raw kernel)

SCHEME = os.environ.get("LSTM_SCHEME", "f16c0")

_compiled = {}
LAST_RESULT = None


def _scheme_cfg(scheme):
    import concourse.mybir as mybir

    if scheme == "fp32":
        return dict(dt=mybir.dt.float32, nw=1, nx=1, pairs=[(0, 0)])
    if scheme == "fp32r":
        return dict(dt=mybir.dt.float32r, nw=1, nx=1, pairs=[(0, 0)])
    if scheme == "bf16x1":
        return dict(dt=mybir.dt.bfloat16, nw=1, nx=1, pairs=[(0, 0)])
    if scheme == "fp16x1":
        return dict(dt=mybir.dt.float16, nw=1, nx=1, pairs=[(0, 0)])
    if scheme == "bf16x3":
        return dict(
            dt=mybir.dt.bfloat16, nw=2, nx=2, pairs=[(0, 0), (1, 0), (0, 1)]
        )
    raise ValueError(scheme)


def _build(scheme):
    import concourse.bacc as bacc
    import concourse.mybir as mybir
    from concourse.tile import TileContext

    cfg = _scheme_cfg(scheme)
    dt_mm = cfg["dt"]
    pairs = cfg["pairs"]
    nsrc = len(pairs)
    f32 = mybir.dt.float32
    AFT = mybir.ActivationFunctionType
    ALU = mybir.AluOpType

    nc = bacc.Bacc("TRN2", target_bir_lowering=False, debug=False)

    w_drams = [
        nc.dram_tensor(f"wt{i}", [HMT, KT * P, 512], dt_mm, kind="ExternalInput")
        for i in range(cfg["nw"])
    ]
    x_drams = [
        nc.dram_tensor(f"xh{i}", [KT * P, B], dt_mm, kind="ExternalInput")
        for i in range(cfg["nx"])
    ]
    cprev_d = nc.dram_tensor("cprev", [RPC, B], f32, kind="ExternalInput")
    bias_d = nc.dram_tensor("bias", [P, HMT * 4], f32, kind="ExternalInput")
    out_d = nc.dram_tensor("out", [P, HMT * NN], f32, kind="ExternalOutput")

    with TileContext(nc) as tc:
        with (
            tc.tile_pool(name="wp", bufs=4) as wp,
            tc.tile_pool(name="xp", bufs=4) as xp,
            tc.tile_pool(name="pp", bufs=8, space="PSUM") as pp,
            tc.tile_pool(name="ep", bufs=2) as ep,
            tc.tile_pool(name="mp", bufs=1) as mp,
        ):
            bias_sb = mp.tile([P, HMT * 4], f32, tag="bias")
            nc.sync.dma_start(out=bias_sb[:, :], in_=bias_d[:, :])
            acc_sb = mp.tile([P, HMT * NN], f32, tag="acc")

            for hm in range(HMT):
                ps = [pp.tile([P, 512], f32, tag="ps", name=f"ps{hm}_{j}") for j in range(8)]
                for kt in range(KT):
                    wt = [
                        wp.tile([P, 512], dt_mm, tag=f"w{i}", name=f"w{i}_{hm}_{kt}")
                        for i in range(cfg["nw"])
                    ]
                    for i in range(cfg["nw"]):
                        nc.sync.dma_start(
                            out=wt[i][:, :],
                            in_=w_drams[i][hm, kt * P : (kt + 1) * P, :],
                        )
                    xt = [
                        xp.tile([P, B], dt_mm, tag=f"x{i}", name=f"x{i}_{hm}_{kt}")
                        for i in range(cfg["nx"])
                    ]
                    for i in range(cfg["nx"]):
                        nc.sync.dma_start(
                            out=xt[i][:, :],
                            in_=x_drams[i][kt * P : (kt + 1) * P, :],
                        )
                    for g in range(4):
                        for si, (wi, xi) in enumerate(pairs):
                            lhsT = wt[wi][:, g * P : (g + 1) * P]
                            for n in range(NN):
                                nc.tensor.matmul(
                                    ps[g * NN + n][:, :],
                                    lhsT,
                                    xt[xi][:, n * 512 : (n + 1) * 512],
                                    start=(kt == 0 and si == 0),
                                    stop=(kt == KT - 1 and si == nsrc - 1),
                                )

                for n in range(NN):
                    idx = hm * NN + n
                    cp = ep.tile([P, 512], f32, tag="cp")
                    nc.sync.dma_start(
                        out=cp[:, :],
                        in_=cprev_d[hm * P : (hm + 1) * P, n * 512 : (n + 1) * 512],
                    )
                    i_sb = ep.tile([P, 512], f32, tag="i")
                    f_sb = ep.tile([P, 512], f32, tag="f")
                    g_sb = ep.tile([P, 512], f32, tag="g")
                    o_sb = ep.tile([P, 512], f32, tag="o")
                    for t_sb, gi, fn in (
                        (i_sb, 0, AFT.Sigmoid),
                        (f_sb, 1, AFT.Sigmoid),
                        (g_sb, 2, AFT.Tanh),
                        (o_sb, 3, AFT.Sigmoid),
                    ):
                        nc.scalar.activation(
                            t_sb[:, :],
                            ps[gi * NN + n][:, :],
                            fn,
                            bias=bias_sb[:, hm * 4 + gi : hm * 4 + gi + 1],
                        )
                    t_fc = ep.tile([P, 512], f32, tag="fc")
                    nc.vector.tensor_mul(t_fc[:, :], f_sb[:, :], cp[:, :])
                    t_ig = ep.tile([P, 512], f32, tag="ig")
                    nc.vector.tensor_mul(t_ig[:, :], i_sb[:, :], g_sb[:, :])
                    t_c = ep.tile([P, 512], f32, tag="c")
                    nc.vector.tensor_add(t_c[:, :], t_fc[:, :], t_ig[:, :])
                    t_tc = ep.tile([P, 512], f32, tag="tc")
                    nc.scalar.activation(t_tc[:, :], t_c[:, :], AFT.Tanh)
                    t_h = ep.tile([P, 512], f32, tag="h")
                    nc.vector.tensor_mul(t_h[:, :], o_sb[:, :], t_tc[:, :])
                    t_s = ep.tile([P, 512], f32, tag="s")
                    nc.vector.tensor_add(t_s[:, :], t_c[:, :], t_h[:, :])
                    nc.vector.reduce_sum(
                        acc_sb[:, idx : idx + 1],
                        t_s[:, :],
                        axis=mybir.AxisListType.X,
                    )

            nc.sync.dma_start(out=out_d[:, :], in_=acc_sb[:, :])

    nc.compile()
    return nc


KT2 = KT // 2  # 32 kt256 tiles for DoubleRow fp8 passes


def _build_f16c(ncorr):
    """fp16 main pass + `ncorr` fp8-DoubleRow correction passes (0/1/2).

    All passes produce products at scale 2^20 so they share one PSUM
    accumulation group per bank (no drain / phase split):
      main:    (W*2^10 as fp16) @ (X*2^10 as fp16)         64 kt, 512 cyc
      corr W:  f8(Wl*2^16) @ f8(X*2^4)   DoubleRow, K=256  32 kt, 256 cyc
      corr X:  f8(W*2^4) @ f8(Xl*2^16)   DoubleRow, K=256  32 kt, 256 cyc
    Epilogue: act(fn, psum * 2^-20 + bias) -- identical cost to baseline.
    W streams on the SP ring, X on the ACT ring; persistents on gpsimd.
    """
    import concourse.bacc as bacc
    import concourse.mybir as mybir
    from concourse.tile import TileContext

    f32 = mybir.dt.float32
    f16 = mybir.dt.float16
    f8 = mybir.dt.float8e4
    AFT = mybir.ActivationFunctionType
    DRM = mybir.MatmulPerfMode.DoubleRow

    nc = bacc.Bacc("TRN2", target_bir_lowering=False, debug=False)

    wh_d = nc.dram_tensor("wh", [HMT, KT * P, 512], f16, kind="ExternalInput")
    xh_d = nc.dram_tensor("xh", [KT * P, B], f16, kind="ExternalInput")
    if ncorr >= 1:
        wl8_d = nc.dram_tensor(
            "wl8", [HMT, KT2 * P, 2, 512], f8, kind="ExternalInput"
        )
        x8_d = nc.dram_tensor("x8", [P, KT2, 2, B], f8, kind="ExternalInput")
    if ncorr >= 2:
        w8_d = nc.dram_tensor(
            "w8", [HMT, KT2 * P, 2, 512], f8, kind="ExternalInput"
        )
        xl8_d = nc.dram_tensor("xl8", [KT2 * P, 2, B], f8, kind="ExternalInput")
    cprev_d = nc.dram_tensor("cprev", [P, HMT * NN, 512], f32, kind="ExternalInput")
    bias_d = nc.dram_tensor("bias", [P, HMT * 4], f32, kind="ExternalInput")
    out_d = nc.dram_tensor("out", [P, HMT * NN], f32, kind="ExternalOutput")

    CSC = float(2.0 ** -20)  # psum descale in the epilogue activations
    GATES = ((0, AFT.Sigmoid), (1, AFT.Sigmoid), (2, AFT.Tanh), (3, AFT.Sigmoid))

    with TileContext(nc) as tc:
        with (
            tc.tile_pool(name="wp", bufs=16) as wp,
            tc.tile_pool(name="xp", bufs=16) as xp,
            tc.tile_pool(name="w8p", bufs=4) as w8p,
            tc.tile_pool(name="x8p", bufs=4) as x8p,
            tc.tile_pool(name="pp", bufs=8, space="PSUM") as pp,
            tc.tile_pool(name="ep", bufs=1) as ep,
            tc.tile_pool(name="mp", bufs=1) as mp,
        ):
            bias_sb = mp.tile([P, HMT * 4], f32, tag="bias")
            nc.gpsimd.dma_start(out=bias_sb[:, :], in_=bias_d[:, :])
            cp_all = mp.tile([P, HMT * NN, 512], f32, tag="cp")
            acc_sb = mp.tile([P, HMT * NN], f32, tag="acc")
            if ncorr >= 1:
                x8_sb = mp.tile([P, KT2, 2, B], f8, tag="x8r")
                nc.gpsimd.dma_start(out=x8_sb[...], in_=x8_d[...])

            def epilogue(hm, ns, ps):
                gates = {}
                for gi, fn in GATES:
                    for n in ns:
                        t_sb = ep.tile(
                            [P, 512], f32, tag=f"g{gi}n{n}", name=f"a{hm}_{gi}_{n}"
                        )
                        nc.scalar.activation(
                            t_sb[:, :],
                            ps[gi * NN + n][:, :],
                            fn,
                            bias=bias_sb[:, hm * 4 + gi : hm * 4 + gi + 1],
                            scale=CSC,
                        )
                        gates[(gi, n)] = t_sb
                tt = {}
                for n in ns:
                    tt[("fc", n)] = ep.tile(
                        [P, 512], f32, tag=f"fc{n}", name=f"fc{n}_{hm}"
                    )
                    nc.vector.tensor_mul(
                        tt[("fc", n)][:, :],
                        gates[(1, n)][:, :],
                        cp_all[:, hm * NN + n, :],
                    )
                for n in ns:
                    tt[("ig", n)] = ep.tile(
                        [P, 512], f32, tag=f"ig{n}", name=f"ig{n}_{hm}"
                    )
                    nc.vector.tensor_mul(
                        tt[("ig", n)][:, :], gates[(0, n)][:, :], gates[(2, n)][:, :]
                    )
                for n in ns:
                    tt[("c", n)] = ep.tile(
                        [P, 512], f32, tag=f"c{n}", name=f"c{n}_{hm}"
                    )
                    nc.vector.tensor_add(
                        tt[("c", n)][:, :], tt[("fc", n)][:, :], tt[("ig", n)][:, :]
                    )
                for n in ns:
                    tt[("tc", n)] = ep.tile(
                        [P, 512], f32, tag=f"tc{n}", name=f"tc{n}_{hm}"
                    )
                    nc.scalar.activation(
                        tt[("tc", n)][:, :], tt[("c", n)][:, :], AFT.Tanh
                    )
                for n in ns:
                    tt[("h", n)] = ep.tile(
                        [P, 512], f32, tag=f"h{n}", name=f"h{n}_{hm}"
                    )
                    nc.vector.tensor_mul(
                        tt[("h", n)][:, :], gates[(3, n)][:, :], tt[("tc", n)][:, :]
                    )
                for n in ns:
                    tt[("s", n)] = ep.tile(
                        [P, 512], f32, tag=f"s{n}", name=f"s{n}_{hm}"
                    )
                    nc.vector.tensor_add(
                        tt[("s", n)][:, :], tt[("c", n)][:, :], tt[("h", n)][:, :]
                    )
                for n in ns:
                    idx = hm * NN + n
                    nc.vector.reduce_sum(
                        acc_sb[:, idx : idx + 1],
                        tt[("s", n)][:, :],
                        axis=mybir.AxisListType.X,
                    )

            for hm in range(HMT):
                ps = [
                    pp.tile([P, 512], f32, tag="ps", name=f"m{hm}_{j}")
                    for j in range(8)
                ]
                if ncorr == 0:
                    # stagger per-bank kt schedules by one step so banks stop
                    # ~1.7us apart: the epilogue pipelines against the PE
                    # wind-down and hm-boundary PSUM WAR bubbles vanish
                    # (per-bank kt order unchanged: 0..63)
                    tiles = {}
                    for s in range(KT + 7):
                        if hm == 0 and s == 24:
                            nc.gpsimd.dma_start(out=cp_all[...], in_=cprev_d[...])
                        if s < KT:
                            wt = wp.tile(
                                [P, 512], f16, tag="w", name=f"w_{hm}_{s}"
                            )
                            (nc.sync if s % 2 == 0 else nc.scalar).dma_start(
                                out=wt[:, :],
                                in_=wh_d[hm, s * P : (s + 1) * P, :],
                            )
                            xt = xp.tile(
                                [P, B], f16, tag="x", name=f"x_{hm}_{s}"
                            )
                            (nc.scalar if s % 2 == 0 else nc.sync).dma_start(
                                out=xt[:, :], in_=xh_d[s * P : (s + 1) * P, :]
                            )
                            tiles[s] = (wt, xt)
                        for b in range(8):
                            kt = s - b
                            if kt < 0 or kt >= KT:
                                continue
                            g, n = divmod(b, NN)
                            wt, xt = tiles[kt]
                            nc.tensor.matmul(
                                ps[b][:, :],
                                wt[:, g * P : (g + 1) * P],
                                xt[:, n * 512 : (n + 1) * 512],
                                start=(kt == 0),
                                stop=(kt == KT - 1),
                            )
                    epilogue(hm, list(range(NN)), ps)
                    continue
                for kt in range(KT):
                    if hm == 0 and kt == 24:
                        # cprev (2MB) deferred past the pipeline warm-up
                        nc.gpsimd.dma_start(out=cp_all[...], in_=cprev_d[...])
                    wt = wp.tile([P, 512], f16, tag="w", name=f"w_{hm}_{kt}")
                    wring = nc.sync if kt % 2 == 0 else nc.scalar
                    xring = nc.scalar if kt % 2 == 0 else nc.sync
                    wring.dma_start(
                        out=wt[:, :], in_=wh_d[hm, kt * P : (kt + 1) * P, :]
                    )
                    xt = xp.tile([P, B], f16, tag="x", name=f"x_{hm}_{kt}")
                    xring.dma_start(
                        out=xt[:, :], in_=xh_d[kt * P : (kt + 1) * P, :]
                    )
                    t = kt // 2
                    corr = None
                    if ncorr == 1 and kt % 2 == 0:
                        wl8 = w8p.tile(
                            [P, 2, 512], f8, tag="wl8", name=f"wl8_{hm}_{t}"
                        )
                        nc.sync.dma_start(
                            out=wl8[:, :, :],
                            in_=wl8_d[hm, t * P : (t + 1) * P, :, :],
                        )
                        corr = (wl8, x8_sb, None)
                    elif ncorr >= 2:
                        if kt % 2 == 0:
                            wl8 = w8p.tile(
                                [P, 2, 512], f8, tag="wl8", name=f"wl8_{hm}_{t}"
                            )
                            nc.sync.dma_start(
                                out=wl8[:, :, :],
                                in_=wl8_d[hm, t * P : (t + 1) * P, :, :],
                            )
                            corr = (wl8, x8_sb, None)
                        else:
                            w8t = w8p.tile(
                                [P, 2, 512], f8, tag="w8", name=f"w8_{hm}_{t}"
                            )
                            nc.sync.dma_start(
                                out=w8t[:, :, :],
                                in_=w8_d[hm, t * P : (t + 1) * P, :, :],
                            )
                            xl8 = x8p.tile(
                                [P, 2, B], f8, tag="xl8", name=f"xl8_{hm}_{t}"
                            )
                            nc.scalar.dma_start(
                                out=xl8[:, :, :],
                                in_=xl8_d[t * P : (t + 1) * P, :, :],
                            )
                            corr = (w8t, None, xl8)
                    last_kt = kt == KT - 1
                    for g in range(4):
                        lhsT = wt[:, g * P : (g + 1) * P]
                        for n in range(NN):
                            # group last writer: f16 for ncorr<=1 (kt63 has
                            # no DR for ncorr=1), the kt63 DR for ncorr=2
                            nc.tensor.matmul(
                                ps[g * NN + n][:, :],
                                lhsT,
                                xt[:, n * 512 : (n + 1) * 512],
                                start=(kt == 0),
                                stop=(ncorr <= 1 and last_kt),
                            )
                            if corr is not None:
                                wt8, xr8, xs8 = corr
                                rhs = (
                                    xr8[:, t, :, n * 512 : (n + 1) * 512]
                                    if xr8 is not None
                                    else xs8[:, :, n * 512 : (n + 1) * 512]
                                )
                                nc.tensor.matmul(
                                    ps[g * NN + n][:, :],
                                    wt8[:, :, g * P : (g + 1) * P],
                                    rhs,
                                    start=False,
                                    stop=(ncorr >= 2 and last_kt),
                                    perf_mode=DRM,
                                )

                epilogue(hm, list(range(NN)), ps)

            nc.gpsimd.dma_start(out=out_d[:, :], in_=acc_sb[:, :])

    nc.compile()
    return nc


def _build_raw_f16(nbuf=16):
    """Hand-scheduled raw-Bass fp16x1 kernel (single pass, scaled 2^20).

    Same engine choreography as _build_raw but one (w, x) stream pair and
    fp16 matmuls; gate activations descale via scale=2^-20.
    """
    import concourse.bacc as bacc
    import concourse.mybir as mybir

    f16 = mybir.dt.float16
    f32 = mybir.dt.float32
    AFT = mybir.ActivationFunctionType

    nc = bacc.Bacc("TRN2", target_bir_lowering=False, debug=False)

    wh_d = nc.dram_tensor("wh", [HMT, KT * P, 512], f16, kind="ExternalInput")
    xh_d = nc.dram_tensor("xh", [KT * P, B], f16, kind="ExternalInput")
    cprev_d = nc.dram_tensor("cprev", [P, HMT * NN, 512], f32, kind="ExternalInput")
    bias_d = nc.dram_tensor("bias", [P, HMT * 4], f32, kind="ExternalInput")
    out_d = nc.dram_tensor("out", [P, HMT * NN], f32, kind="ExternalOutput")

    NG = HMT * KT  # 256 k-tile groups
    CSC = float(2.0 ** -20)

    from contextlib import ExitStack

    with ExitStack() as ctx:
        e = ctx.enter_context
        wsb = e(nc.sbuf_tensor([P, nbuf, 512], f16))
        xsb = e(nc.sbuf_tensor([P, nbuf, 1024], f16))
        cp = e(nc.sbuf_tensor([P, HMT * NN, 512], f32))
        bias_sb = e(nc.sbuf_tensor([P, HMT * 4], f32))
        acc_sb = e(nc.sbuf_tensor([P, HMT * NN], f32))
        ps = e(nc.psum_tensor([P, 8, 512], f32))
        i_sb = e(nc.sbuf_tensor([P, NN, 512], f32))
        f_sb = e(nc.sbuf_tensor([P, NN, 512], f32))
        g_sb = e(nc.sbuf_tensor([P, NN, 512], f32))
        o_sb = e(nc.sbuf_tensor([P, NN, 512], f32))
        t_fc = e(nc.sbuf_tensor([P, NN, 512], f32))
        t_ig = e(nc.sbuf_tensor([P, NN, 512], f32))
        t_c = e(nc.sbuf_tensor([P, NN, 512], f32))
        t_tc = e(nc.sbuf_tensor([P, NN, 512], f32))
        t_h = e(nc.sbuf_tensor([P, NN, 512], f32))
        t_s = e(nc.sbuf_tensor([P, NN, 512], f32))
        dsems = [e(nc.semaphore(f"dsem{j}")) for j in range(nbuf)]
        init_sem = e(nc.semaphore("init_sem"))
        pe_bank = e(nc.semaphore("pe_bank"))
        pe_kt = e(nc.semaphore("pe_kt"))
        a_sem = e(nc.semaphore("a_sem"))
        d_sem = e(nc.semaphore("d_sem"))
        block = e(nc.Block(no_gpsimd_drain=True))

        DMA_INIT = 2  # bias + cprev

        @block.sync
        def _(sync):
            def init_dmas():
                sync.dma_start(out=bias_sb[:, :], in_=bias_d[:, :]).then_inc(
                    init_sem, 16
                )
                sync.dma_start(out=cp[:, :, :], in_=cprev_d[:, :, :]).then_inc(
                    init_sem, 16
                )

            for gi in range(NG):
                if gi == nbuf:
                    init_dmas()
                hm, kt = divmod(gi, KT)
                slot = gi % nbuf
                if gi >= nbuf:
                    sync.wait_ge(pe_kt, gi - nbuf + 1)
                sync.dma_start(
                    out=wsb[:, slot, :],
                    in_=wh_d[hm, kt * P : (kt + 1) * P, :],
                ).then_inc(dsems[slot], 16)
                if gi >= nbuf:
                    sync.dma_start(
                        out=xsb[:, slot, :],
                        in_=xh_d[kt * P : (kt + 1) * P, :],
                    ).then_inc(dsems[slot], 16)
            sync.wait_ge(d_sem, 12 * HMT)
            sync.dma_start(out=out_d[:, :], in_=acc_sb[:, :]).then_inc(init_sem, 16)

        @block.tensor
        def _(tensor):
            for hm in range(HMT):
                for kt in range(KT):
                    gi = hm * KT + kt
                    slot = gi % nbuf
                    tensor.wait_ge(dsems[slot], 32 * (gi // nbuf + 1))
                    mm = None
                    for g in range(4):
                        lhsT = wsb[:, slot, g * P : (g + 1) * P]
                        for n in range(NN):
                            if kt == 0 and hm > 0:
                                v = 10 * (hm - 1) + 2 * g + n + 1
                                tensor.wait_ge(a_sem, v)
                            mm = nc.tensor.matmul(
                                ps[:, g * NN + n, :],
                                lhsT,
                                xsb[:, slot, n * 512 : (n + 1) * 512],
                                start=(kt == 0),
                                stop=(kt == KT - 1),
                            )
                            if kt == KT - 1 and not (g == 3 and n == NN - 1):
                                mm.then_inc(pe_bank, 1)
                    mm.then_inc(pe_kt, 1)

        @block.scalar
        def _(scalar):
            for gi in range(nbuf):
                scalar.dma_start(
                    out=xsb[:, gi, :], in_=xh_d[gi * P : (gi + 1) * P, :]
                ).then_inc(dsems[gi], 16)
            scalar.wait_ge(init_sem, 16 * DMA_INIT)  # bias loaded
            war = {0: (2, 5), 1: (1, 4), 2: (2, 5), 3: (7, 10)}
            for hm in range(HMT):
                for g, (t_sb, fn) in enumerate(
                    (
                        (i_sb, AFT.Sigmoid),
                        (f_sb, AFT.Sigmoid),
                        (g_sb, AFT.Tanh),
                        (o_sb, AFT.Sigmoid),
                    )
                ):
                    for n in range(NN):
                        if hm > 0:
                            scalar.wait_ge(d_sem, 12 * (hm - 1) + war[g][n])
                        if g == 3 and n == NN - 1:
                            scalar.wait_ge(pe_kt, KT * (hm + 1))
                        else:
                            scalar.wait_ge(pe_bank, 7 * hm + 2 * g + n + 1)
                        nc.scalar.activation(
                            t_sb[:, n, :],
                            ps[:, g * NN + n, :],
                            fn,
                            bias=bias_sb[:, hm * 4 + g : hm * 4 + g + 1],
                            scale=CSC,
                        ).then_inc(a_sem, 1)
                for n in range(NN):
                    scalar.wait_ge(d_sem, 12 * hm + (3 if n == 0 else 6))
                    nc.scalar.activation(
                        t_tc[:, n, :], t_c[:, n, :], AFT.Tanh
                    ).then_inc(a_sem, 1)

        @block.vector
        def _(vector):
            vector.wait_ge(init_sem, 16 * DMA_INIT)  # cprev loaded
            for hm in range(HMT):
                base = 10 * hm
                for n in range(NN):
                    vector.wait_ge(a_sem, base + (3 if n == 0 else 4))
                    nc.vector.tensor_mul(
                        t_fc[:, n, :], f_sb[:, n, :], cp[:, hm * NN + n, :]
                    ).then_inc(d_sem, 1)
                    vector.wait_ge(a_sem, base + (5 if n == 0 else 6))
                    nc.vector.tensor_mul(
                        t_ig[:, n, :], i_sb[:, n, :], g_sb[:, n, :]
                    ).then_inc(d_sem, 1)
                    nc.vector.tensor_add(
                        t_c[:, n, :], t_fc[:, n, :], t_ig[:, n, :]
                    ).then_inc(d_sem, 1)
                for n in range(NN):
                    vector.wait_ge(a_sem, base + (9 if n == 0 else 10))
                    nc.vector.tensor_mul(
                        t_h[:, n, :], o_sb[:, n, :], t_tc[:, n, :]
                    ).then_inc(d_sem, 1)
                    nc.vector.tensor_add(
                        t_s[:, n, :], t_c[:, n, :], t_h[:, n, :]
                    ).then_inc(d_sem, 1)
                    idx = hm * NN + n
                    nc.vector.reduce_sum(
                        acc_sb[:, idx : idx + 1],
                        t_s[:, n, :],
                        axis=mybir.AxisListType.X,
                    ).then_inc(d_sem, 1)

    nc.compile()
    return nc


def _build_raw():
    import concourse.bacc as bacc
    import concourse.mybir as mybir

    dt = mybir.dt.bfloat16
    f32 = mybir.dt.float32
    AFT = mybir.ActivationFunctionType
    ALU = mybir.AluOpType

    nc = bacc.Bacc("TRN2", target_bir_lowering=False, debug=False)

    w_drams = [
        nc.dram_tensor(f"wt{i}", [HMT, KT * P, 512], dt, kind="ExternalInput")
        for i in range(2)
    ]
    x_drams = [
        nc.dram_tensor(f"xh{i}", [KT * P, B], dt, kind="ExternalInput")
        for i in range(2)
    ]
    cprev_d = nc.dram_tensor("cprev", [RPC, B], f32, kind="ExternalInput")
    bias_d = nc.dram_tensor("bias", [P, HMT * 4], f32, kind="ExternalInput")
    out_d = nc.dram_tensor("out", [P, HMT * NN], f32, kind="ExternalOutput")

    NG = HMT * KT  # 256 k-tile groups

    from contextlib import ExitStack

    with ExitStack() as ctx:
        e = ctx.enter_context
        whi = e(nc.sbuf_tensor([P, NBUF, 512], dt))
        wlo = e(nc.sbuf_tensor([P, NBUF, 512], dt))
        xhi = e(nc.sbuf_tensor([P, NBUF, 1024], dt))
        xlo = e(nc.sbuf_tensor([P, NBUF, 1024], dt))
        cp = e(nc.sbuf_tensor([P, HMT * NN, 512], f32))
        bias_sb = e(nc.sbuf_tensor([P, HMT * 4], f32))
        acc_sb = e(nc.sbuf_tensor([P, HMT * NN], f32))
        ps = e(nc.psum_tensor([P, 8, 512], f32))
        i_sb = e(nc.sbuf_tensor([P, NN, 512], f32))
        f_sb = e(nc.sbuf_tensor([P, NN, 512], f32))
        g_sb = e(nc.sbuf_tensor([P, NN, 512], f32))
        o_sb = e(nc.sbuf_tensor([P, NN, 512], f32))
        t_fc = e(nc.sbuf_tensor([P, NN, 512], f32))
        t_ig = e(nc.sbuf_tensor([P, NN, 512], f32))
        t_c = e(nc.sbuf_tensor([P, NN, 512], f32))
        t_tc = e(nc.sbuf_tensor([P, NN, 512], f32))
        t_h = e(nc.sbuf_tensor([P, NN, 512], f32))
        t_s = e(nc.sbuf_tensor([P, NN, 512], f32))
        dsems = [e(nc.semaphore(f"dsem{j}")) for j in range(NBUF)]
        init_sem = e(nc.semaphore("init_sem"))
        pe_bank = e(nc.semaphore("pe_bank"))
        pe_kt = e(nc.semaphore("pe_kt"))
        a_sem = e(nc.semaphore("a_sem"))
        d_sem = e(nc.semaphore("d_sem"))
        block = e(nc.Block(no_gpsimd_drain=True))

        DMA_INIT = 1 + HMT * NN  # bias + cprev tiles

        @block.sync
        def _(sync):
            def init_dmas():
                sync.dma_start(out=bias_sb[:, :], in_=bias_d[:, :]).then_inc(
                    init_sem, 16
                )
                for hm in range(HMT):
                    for n in range(NN):
                        sync.dma_start(
                            out=cp[:, hm * NN + n, :],
                            in_=cprev_d[
                                hm * P : (hm + 1) * P, n * 512 : (n + 1) * 512
                            ],
                        ).then_inc(init_sem, 16)

            for gi in range(NG):
                if gi == NBUF:
                    # init tensors aren't needed until the first epilogue;
                    # issue them after the stream pipeline is primed so kt0
                    # isn't stuck behind 9 serial ring transfers.
                    init_dmas()
                hm, kt = divmod(gi, KT)
                slot = gi % NBUF
                if gi >= NBUF:
                    sync.wait_ge(pe_kt, gi - NBUF + 1)
                sync.dma_start(
                    out=whi[:, slot, :],
                    in_=w_drams[0][hm, kt * P : (kt + 1) * P, :],
                ).then_inc(dsems[slot], 16)
                sync.dma_start(
                    out=wlo[:, slot, :],
                    in_=w_drams[1][hm, kt * P : (kt + 1) * P, :],
                ).then_inc(dsems[slot], 16)
                if gi >= NBUF:
                    sync.dma_start(
                        out=xhi[:, slot, :],
                        in_=x_drams[0][kt * P : (kt + 1) * P, :],
                    ).then_inc(dsems[slot], 16)
                    sync.dma_start(
                        out=xlo[:, slot, :],
                        in_=x_drams[1][kt * P : (kt + 1) * P, :],
                    ).then_inc(dsems[slot], 16)
            sync.wait_ge(d_sem, 12 * HMT)
            sync.dma_start(out=out_d[:, :], in_=acc_sb[:, :]).then_inc(init_sem, 16)

        @block.tensor
        def _(tensor):
            for hm in range(HMT):
                for kt in range(KT):
                    gi = hm * KT + kt
                    slot = gi % NBUF
                    # slot-sem threshold: use-(gi//NBUF) of this slot fully
                    # DMA'd.  Unambiguous even with unordered DMA completion:
                    # the next use of this slot is issued only after SP's WAR
                    # wait on pe_kt, which itself requires this wait to pass.
                    tensor.wait_ge(dsems[slot], 64 * (gi // NBUF + 1))
                    mm = None
                    for g in range(4):
                        for si, (wt, xt) in enumerate(
                            ((whi, xhi), (wlo, xhi), (whi, xlo))
                        ):
                            lhsT = wt[:, slot, g * P : (g + 1) * P]
                            for n in range(NN):
                                if kt == 0 and si == 0 and hm > 0:
                                    # per-bank WAR wait: ACT of prev phase must
                                    # have read this bank (g-major act order:
                                    # inc 2g + n + 1)
                                    v = 10 * (hm - 1) + 2 * g + n + 1
                                    tensor.wait_ge(a_sem, v)
                                mm = nc.tensor.matmul(
                                    ps[:, g * NN + n, :],
                                    lhsT,
                                    xt[:, slot, n * 512 : (n + 1) * 512],
                                    start=(kt == 0 and si == 0),
                                    stop=(kt == KT - 1 and si == 2),
                                )
                                if (
                                    kt == KT - 1
                                    and si == 2
                                    and not (g == 3 and n == NN - 1)
                                ):
                                    # bank (g, n) fully accumulated: let ACT
                                    # start this gate's activation while the
                                    # remaining banks still stream.  The very
                                    # last bank signals via pe_kt instead (a
                                    # MM can carry only one sem update).
                                    mm.then_inc(pe_bank, 1)
                    mm.then_inc(pe_kt, 1)

        @block.scalar
        def _(scalar):
            # first NBUF groups' x tiles go out on the ACT HWDGE ring so the
            # startup isn't serialized behind SP's w DMAs (slot-sem thresholds
            # are ring-agnostic: 4 incs of 16 per slot either way)
            for gi in range(NBUF):
                scalar.dma_start(
                    out=xhi[:, gi, :], in_=x_drams[0][gi * P : (gi + 1) * P, :]
                ).then_inc(dsems[gi], 16)
                scalar.dma_start(
                    out=xlo[:, gi, :], in_=x_drams[1][gi * P : (gi + 1) * P, :]
                ).then_inc(dsems[gi], 16)
            scalar.wait_ge(init_sem, 16 * DMA_INIT)  # bias loaded
            # WAR thresholds: last phase-(hm-1) DVE reader of each gate tile
            # (i: ig, f: fc, g: ig, o: h) per n half
            war = {0: (2, 5), 1: (1, 4), 2: (2, 5), 3: (7, 10)}
            for hm in range(HMT):
                # 8 gate activations in gate-major order -- matches both the
                # stop-MM inc order of the last k-tile AND the bank-touch
                # order of the next phase's kt0, so banks free just in time.
                # Then 2 tanh(c).
                for g, (t_sb, fn) in enumerate(
                    (
                        (i_sb, AFT.Sigmoid),
                        (f_sb, AFT.Sigmoid),
                        (g_sb, AFT.Tanh),
                        (o_sb, AFT.Sigmoid),
                    )
                ):
                    for n in range(NN):
                        if hm > 0:
                            scalar.wait_ge(d_sem, 12 * (hm - 1) + war[g][n])
                        if g == 3 and n == NN - 1:
                            scalar.wait_ge(pe_kt, KT * (hm + 1))
                        else:
                            scalar.wait_ge(pe_bank, 7 * hm + 2 * g + n + 1)
                        nc.scalar.activation(
                            t_sb[:, n, :],
                            ps[:, g * NN + n, :],
                            fn,
                            bias=bias_sb[:, hm * 4 + g : hm * 4 + g + 1],
                        ).then_inc(a_sem, 1)
                for n in range(NN):
                    scalar.wait_ge(d_sem, 12 * hm + (3 if n == 0 else 6))
                    nc.scalar.activation(
                        t_tc[:, n, :], t_c[:, n, :], AFT.Tanh
                    ).then_inc(a_sem, 1)

        @block.vector
        def _(vector):
            vector.wait_ge(init_sem, 16 * DMA_INIT)  # cprev tiles loaded
            for hm in range(HMT):
                base = 10 * hm
                for n in range(NN):
                    vector.wait_ge(a_sem, base + (3 if n == 0 else 4))
                    nc.vector.tensor_mul(
                        t_fc[:, n, :], f_sb[:, n, :], cp[:, hm * NN + n, :]
                    ).then_inc(d_sem, 1)
                    vector.wait_ge(a_sem, base + (5 if n == 0 else 6))
                    nc.vector.tensor_mul(
                        t_ig[:, n, :], i_sb[:, n, :], g_sb[:, n, :]
                    ).then_inc(d_sem, 1)
                    nc.vector.tensor_add(
                        t_c[:, n, :], t_fc[:, n, :], t_ig[:, n, :]
                    ).then_inc(d_sem, 1)
                for n in range(NN):
                    vector.wait_ge(a_sem, base + (9 if n == 0 else 10))
                    nc.vector.tensor_mul(
                        t_h[:, n, :], o_sb[:, n, :], t_tc[:, n, :]
                    ).then_inc(d_sem, 1)
                    nc.vector.tensor_add(
                        t_s[:, n, :], t_c[:, n, :], t_h[:, n, :]
                    ).then_inc(d_sem, 1)
                    idx = hm * NN + n
                    nc.vector.reduce_sum(
                        acc_sb[:, idx : idx + 1],
                        t_s[:, n, :],
                        axis=mybir.AxisListType.X,
                    ).then_inc(d_sem, 1)

    nc.compile()
    return nc



def _get_compiled(scheme):
    if scheme not in _compiled:
        if scheme == "bf16x3":
            _compiled[scheme] = _build_raw()
        elif scheme == "bf16x3_tile":
            _compiled[scheme] = _build("bf16x3")
        elif scheme.startswith("f16c"):
            _compiled[scheme] = _build_f16c(int(scheme[4]))
        elif scheme == "f16r":
            _compiled[scheme] = _build_raw_f16()
        else:
            _compiled[scheme] = _build(scheme)
    return _compiled[scheme]


def _split_lohi(a, np_dt):
    hi = a.astype(np_dt)
    lo = (a - hi.astype(np.float32)).astype(np_dt)
    return hi, lo


def _prep_inputs(scheme, x, h_prev, c_prev, w_ih, w_hh, b_ih, b_hh):
    import ml_dtypes

    f32 = np.float32
    x = np.asarray(x, f32)
    h_prev = np.asarray(h_prev, f32)
    c_prev = np.asarray(c_prev, f32)
    w_ih = np.asarray(w_ih, f32)
    w_hh = np.asarray(w_hh, f32)
    b = (np.asarray(b_ih, f32) + np.asarray(b_hh, f32)).reshape(4, NCORES, HMT, P)

    xh = np.concatenate([x, h_prev], axis=0)  # [8192, B]

    if scheme == "bf16x3_tile":
        scheme = "bf16x3"
    if scheme in ("fp32", "fp32r"):
        np_dt = f32
    elif scheme in ("bf16x1", "bf16x3"):
        np_dt = ml_dtypes.bfloat16
    elif scheme == "fp16x1":
        np_dt = np.float16
    else:
        raise ValueError(scheme)

    split = scheme.endswith("x3")
    if split:
        xh_hi, xh_lo = _split_lohi(xh, np_dt)
        x_maps = {"xh0": xh_hi, "xh1": xh_lo}
    else:
        x_maps = {"xh0": xh.astype(np_dt)}

    wih_r = w_ih.reshape(4, NCORES, RPC, D)
    whh_r = w_hh.reshape(4, NCORES, RPC, H)

    in_maps = []
    for d in range(NCORES):
        wc = np.concatenate([wih_r[:, d], whh_r[:, d]], axis=2)  # (4, 512, 8192)
        wc = wc.reshape(4, HMT, P, D + H)  # (g, hm, r, k)
        wt = np.ascontiguousarray(wc.transpose(1, 3, 0, 2)).reshape(
            HMT, D + H, 4 * P
        )  # (hm, k, g*128+r)
        m = dict(x_maps)
        if split:
            w_hi, w_lo = _split_lohi(wt, np_dt)
            m["wt0"] = w_hi
            m["wt1"] = w_lo
        else:
            m["wt0"] = wt.astype(np_dt)
        m["cprev"] = np.ascontiguousarray(c_prev[d * RPC : (d + 1) * RPC])
        m["bias"] = np.ascontiguousarray(
            b[:, d].transpose(2, 1, 0).reshape(P, HMT * 4)
        )
        in_maps.append(m)
    return in_maps


def _prep_inputs_f16c(ncorr, x, h_prev, c_prev, w_ih, w_hh, b_ih, b_hh):
    """Host prep for the f16c* schemes (fp16 main @ 2^10/side + f8 corr)."""
    import ml_dtypes

    f32 = np.float32
    e4 = ml_dtypes.float8_e4m3
    S10 = f32(2.0 ** 10)
    S4 = f32(2.0 ** 4)
    S16 = f32(2.0 ** 16)

    x = np.asarray(x, f32)
    h_prev = np.asarray(h_prev, f32)
    c_prev = np.asarray(c_prev, f32)
    b = (np.asarray(b_ih, f32) + np.asarray(b_hh, f32)).reshape(4, NCORES, HMT, P)

    X = np.concatenate([x, h_prev], axis=0)          # [8192, B] fp32
    Xh16 = (X * S10).astype(np.float16)              # main rhs (scaled 2^10)
    x_maps = {"xh": Xh16}
    if ncorr >= 1:
        # resident f8 rhs layout [p, t, s, b] with k = 256 t + 128 s + p
        x_maps["x8"] = np.ascontiguousarray(
            (X * S4).astype(e4).reshape(KT2, 2, P, B).transpose(2, 0, 1, 3)
        )
    if ncorr >= 2:
        # streamed f8 rhs layout [t*128+p, s, b]
        Xl = X - Xh16.astype(f32) / S10
        x_maps["xl8"] = np.ascontiguousarray(
            (Xl * S16).astype(e4).reshape(KT2, 2, P, B).transpose(0, 2, 1, 3)
        ).reshape(KT2 * P, 2, B)

    W = np.concatenate(
        [np.asarray(w_ih, f32), np.asarray(w_hh, f32)], axis=1
    )                                                 # [4H, D+H] fp32
    Wh16 = (W * S10).astype(np.float16)
    if ncorr >= 1:
        Wl8 = ((W - Wh16.astype(f32) / S10) * S16).astype(e4)
    if ncorr >= 2:
        W8 = (W * S4).astype(e4)

    def wmain_layout(v, d):
        # v [4H, K] -> per-core [HMT, K, 4*128] with col = g*128+r
        wc = v.reshape(4, NCORES, HMT, P, D + H)[:, d]   # (g, hm, r, k)
        return np.ascontiguousarray(wc.transpose(1, 3, 0, 2)).reshape(
            HMT, D + H, 4 * P
        )

    def wcorr_layout(v, d):
        # v [4H, K] -> per-core [HMT, KT2*P, 2, 512], k = 256 t + 128 s + p
        wc = v.reshape(4, NCORES, HMT, P, KT2, 2, P)[:, d]  # (g,hm,r,t,s,p)
        return np.ascontiguousarray(wc.transpose(1, 3, 5, 4, 0, 2)).reshape(
            HMT, KT2 * P, 2, 4 * P
        )

    in_maps = []
    for d in range(NCORES):
        m = dict(x_maps)
        m["wh"] = wmain_layout(Wh16, d)
        if ncorr >= 1:
            m["wl8"] = wcorr_layout(Wl8, d)
        if ncorr >= 2:
            m["w8"] = wcorr_layout(W8, d)
        m["cprev"] = np.ascontiguousarray(
            c_prev[d * RPC : (d + 1) * RPC]
            .reshape(HMT, P, NN, 512)
            .transpose(1, 0, 2, 3)
        ).reshape(P, HMT * NN, 512)
        m["bias"] = np.ascontiguousarray(
            b[:, d].transpose(2, 1, 0).reshape(P, HMT * 4)
        )
        in_maps.append(m)
    return in_maps


def _ensure_axon_ntff_hook():
    """Register the axon NTFF-profile hook if the container's `antenv` stub
    lacks `axon_hooks` (needed only for trace=True / BASS_TRACE runs)."""
    import contextlib
    import ctypes
    import sys
    import types

    try:
        from antenv import axon_hooks  # noqa: F401

        return
    except ImportError:
        pass
    try:
        import antenv
    except ImportError:
        return

    holder = {}
    mod = types.ModuleType("antenv.axon_hooks")
    mod.set_axon_ntff_profile_hook = lambda h: holder.__setitem__("h", h)
    mod.get_axon_ntff_profile_hook = lambda: holder.get("h")
    sys.modules["antenv.axon_hooks"] = mod
    antenv.axon_hooks = mod

    so_path = "/opt/axon/libaxon_pjrt.so"
    try:
        lib = ctypes.CDLL(so_path)
        if not hasattr(lib, "axon_start_nrt_profile"):
            return
        lib.axon_start_nrt_profile.argtypes = [
            ctypes.POINTER(ctypes.c_int64),
            ctypes.c_size_t,
        ]
        lib.axon_start_nrt_profile.restype = ctypes.c_int64
        lib.axon_stop_nrt_profile.argtypes = [ctypes.c_char_p]
        lib.axon_stop_nrt_profile.restype = ctypes.c_int64

        @contextlib.contextmanager
        def _hook(output_dir, device_ids):
            import jax

            jax.devices()
            if device_ids:
                ids = (ctypes.c_int64 * len(device_ids))(*device_ids)
                rc = lib.axon_start_nrt_profile(ids, len(device_ids))
            else:
                rc = lib.axon_start_nrt_profile(None, 0)
            if rc != 0:
                raise RuntimeError(f"axon_start_nrt_profile rc={rc}")
            try:
                yield
            finally:
                n = lib.axon_stop_nrt_profile(str(output_dir).encode())
                print(f"ntff profile: {n} file(s) -> {output_dir}", file=sys.stderr)

        mod.set_axon_ntff_profile_hook(_hook)
    except Exception:
        pass


def kernel(x, h_prev, c_prev, w_ih, w_hh, b_ih, b_hh):
    global LAST_RESULT
    from concourse.bass_utils import run_bass_kernel_spmd

    if os.environ.get("BASS_TRACE"):
        _ensure_axon_ntff_hook()

    scheme = SCHEME
    nc = _get_compiled(scheme)
    if scheme.startswith("f16c") or scheme == "f16r":
        ncorr = 0 if scheme == "f16r" else int(scheme[4])
        in_maps = _prep_inputs_f16c(
            ncorr, x, h_prev, c_prev, w_ih, w_hh, b_ih, b_hh
        )
    else:
        in_maps = _prep_inputs(scheme, x, h_prev, c_prev, w_ih, w_hh, b_ih, b_hh)
    res = run_bass_kernel_spmd(nc, in_maps, core_ids=list(range(NCORES)))
    LAST_RESULT = res
    total = np.float64(0.0)
    for r in res.results:
        total += np.asarray(r["out"], np.float64).sum()
    return np.array(total, dtype=np.float32)

